# revision 8
# baseline (speedup 1.0000x reference)
"""CRF forward (logZ) + Viterbi decode kernel for Trainium2, 8 NeuronCores.

Problem: B=512, T=1024, K=48 (START=46, END=47), feats [B,T,K] f32,
transitions [K,K] f32. Outputs (logz [B], scores [B], paths [B,T] int32)
matching the jax reference (paths/scores bit-exactly, logz to ~1e-6 rel).

Sharding: data-parallel over batch, 64 sequences per core.

Per-core layout (n=64 sequences):
  Viterbi state v lives in PSUM as v_full[128, 48] with partition p = 2b+h
  (h in {0,1} splits the 48 'next-tag' axis in halves of 24) and per-parity
  rotated columns: v_full[2b+h, c] = v[b, (c+24h) % 48].
  Each step:
    P1 (DVE):  scores[p,(i,c)] = T_rep[p,(i,c)] + v_full[p,c]   [128 x 1152]
    P2 (DVE):  vmax[p,i] = max_c scores                          -> [128, 24]
    PE: v_full[:,0:24]  = I    @ vmax (+ I @ featA_t accumulate)
        v_full[:,24:48] = XOR1 @ vmax (+ XOR1 @ featA_t)        (fp32, exact)
  (fp32 matmul with 0/1 weights is bitwise-exact passthrough on TRN2 - verified)
  logZ rides along on PE/ACT/GPS in an exp-domain normalized recursion.
  Viterbi backtrace re-reads stored v_t from DRAM and chases argmax
  backwards with a onehot-matmul row-gather of the transition matrix
  (bf16 3-split => exact f32 rows).
"""

import numpy as np

B, T, K = 512, 1024, 48
START, END = 46, 47
NEG = -10000.0
NCORES = 8
NPC = B // NCORES          # 64 sequences per core
CH = 64                    # time steps per For_i chunk
NCHUNK = T // CH
NORM_EVERY = 4             # logz renormalization cadence

_CACHE = {}


def _consts(transitions):
    import ml_dtypes

    tb = np.array(transitions, dtype=np.float32).copy()
    tb[START, :] = NEG
    tb[:, END] = NEG

    # T_rep[2b+h, i, c] = tb[24h+i, (c+24h)%48]  (independent of b)
    t_rep = np.zeros((128, 24, 48), np.float32)
    for h in range(2):
        rows = tb[24 * h:24 * h + 24, :]                      # [24, 48]
        rot = np.roll(rows, -24 * h, axis=1)                  # col c -> j=(c+24h)%48
        t_rep[h::2, :, :] = rot[None, :, :]
    t_rep = t_rep.reshape(128, 24 * 48)

    ident = np.eye(128, dtype=np.float32)
    xor1 = np.zeros((128, 128), np.float32)
    for m in range(128):
        xor1[m ^ 1, m] = 1.0
    # featA is stored h-major (row = 64h + b); these route it to p=2b+h rows
    pfi = np.zeros((128, 128), np.float32)
    pfx = np.zeros((128, 128), np.float32)
    for m in range(128):
        b, h = m // 2, m % 2
        pfi[64 * h + b, m] = 1.0          # own half
        pfx[64 * (1 - h) + b, m] = 1.0    # partner half
    i64 = np.eye(64, dtype=np.float32)
    identb = np.eye(128, dtype=ml_dtypes.bfloat16)

    # v init: init[j] = NEG except START -> 0; v_init[2b+h, c] = init[(c+24h)%48]
    init = np.full(48, NEG, np.float32)
    init[START] = 0.0
    v_init = np.zeros((128, 48), np.float32)
    for h in range(2):
        v_init[h::2, :] = np.roll(init, -24 * h)[None, :]

    # logz: EA[j, m]: m<48 -> exp(tb[m, j]); m=48 -> colsum_j
    eh = np.exp(tb.astype(np.float64)).astype(np.float32)     # [i, j]
    ea = np.zeros((48, 65), np.float32)
    ea[:, :48] = eh.T
    ea[:, 64] = eh.sum(0)          # sum row lands at partition 64 (aligned)
    eaf = np.zeros((48, 65), np.float32)
    eaf[:, 64] = eh[END, :]        # final END-row dot at partition 64
    e0 = np.zeros((48, NPC), np.float32)
    e0[START, :] = 1.0

    # backward consts
    hi = tb.astype(ml_dtypes.bfloat16).astype(np.float32)
    mid = (tb - hi).astype(ml_dtypes.bfloat16).astype(np.float32)
    lo = (tb - hi - mid).astype(ml_dtypes.bfloat16).astype(np.float32)
    assert np.all(hi + mid + lo == tb)
    iota48 = np.broadcast_to(np.arange(48, dtype=np.float32), (NPC, 48)).copy()
    t47rep = np.broadcast_to(tb[END], (NPC, 48)).copy()

    return {
        "t_rep": t_rep, "ident": ident, "xor1": xor1, "i64": i64,
        "pfi": pfi, "pfx": pfx,
        "identb": np.asarray(identb), "v_init": v_init, "ea": ea, "eaf": eaf, "e0": e0,
        "t_hi": np.asarray(hi.astype(ml_dtypes.bfloat16)),
        "t_mid": np.asarray(mid.astype(ml_dtypes.bfloat16)),
        "t_lo": np.asarray(lo.astype(ml_dtypes.bfloat16)),
        "iota48": iota48, "t47rep": t47rep,
    }


def _build(t_total=T, ch=CH):
    import concourse.bass as bass
    import concourse.bacc as bacc
    import concourse.mybir as mybir
    import concourse.tile as tile

    F32 = mybir.dt.float32
    BF16 = mybir.dt.bfloat16
    U32 = mybir.dt.uint32
    AX = mybir.AxisListType
    OP = mybir.AluOpType
    ACTF = mybir.ActivationFunctionType
    nchunk = t_total // ch

    nc = bacc.Bacc(None)
    D = {}
    D["feats"] = nc.dram_tensor("feats", [NPC, t_total, K], F32,
                                kind="ExternalInput")
    for nme, shp, dt in [("t_rep", [128, 24 * 48], F32), ("ident", [128, 128], F32),
                         ("xor1", [128, 128], F32), ("i64", [64, 64], F32),
                         ("pfi", [128, 128], F32), ("pfx", [128, 128], F32),
                         ("identb", [128, 128], BF16), ("v_init", [128, 48], F32),
                         ("ea", [48, 65], F32), ("eaf", [48, 65], F32),
                         ("e0", [48, NPC], F32),
                         ("t_hi", [48, 48], BF16), ("t_mid", [48, 48], BF16),
                         ("t_lo", [48, 48], BF16), ("iota48", [NPC, 48], F32),
                         ("t47rep", [NPC, 48], F32)]:
        D[nme] = nc.dram_tensor(nme, shp, dt, kind="ExternalInput")

    D["logz"] = nc.dram_tensor("logz", [1, NPC], F32, kind="ExternalOutput")
    D["scores"] = nc.dram_tensor("scores", [NPC, 1], F32, kind="ExternalOutput")
    D["paths"] = nc.dram_tensor("paths", [NPC, t_total], U32, kind="ExternalOutput")
    import os as _os
    _dbg = _os.environ.get("KM_DEBUG") == "1"
    if _dbg:
        D["dbg_sc"] = nc.dram_tensor("dbg_sc", [t_total, NPC, 48], F32,
                                     kind="ExternalOutput")
        D["dbg_tag"] = nc.dram_tensor("dbg_tag", [t_total, NPC], F32,
                                      kind="ExternalOutput")
    v_dram = nc.dram_tensor("v_all", [t_total, NPC, 48], F32)

    def ap_of(t):
        return t[:] if not isinstance(t, bass.AP) else t

    def apx(t, dims, extra_off=0):
        a = ap_of(t)
        return bass.AP(tensor=a.tensor, offset=a.offset + extra_off,
                       ap=[a.ap[0]] + dims)

    with tile.TileContext(nc) as tc:
        with tc.tile_pool(name="const", bufs=1) as cpool, \
             tc.tile_pool(name="state", bufs=1) as spool, \
             tc.tile_pool(name="pstate", bufs=1, space="PSUM") as pspool, \
             tc.tile_pool(name="work", bufs=2) as wpool, \
             tc.tile_pool(name="feat", bufs=2) as fpool:

            # ---- load constants (stage PE-consumed ones through ACT) ----
            def loadc(nme, dt=F32, act_stage=False):
                shp = list(D[nme].shape)
                t = cpool.tile(shp, dt, tag="ld_" + nme, name="ld_" + nme)
                nc.sync.dma_start(out=t[:], in_=D[nme][:])
                if act_stage:
                    t2 = cpool.tile(shp, dt, tag="st_" + nme, name="st_" + nme)
                    nc.scalar.copy(t2[:], t[:])
                    return t2
                return t

            c_trep = loadc("t_rep")                      # DVE in0
            c_ident = loadc("ident", act_stage=True)     # PE
            c_xor1 = loadc("xor1", act_stage=True)       # PE
            c_pfi = loadc("pfi", act_stage=True)
            c_pfx = loadc("pfx", act_stage=True)
            c_i64 = loadc("i64", act_stage=True)         # PE
            c_identb = loadc("identb", BF16, act_stage=True)
            c_vinit = loadc("v_init")
            c_ea = loadc("ea", act_stage=True)
            c_eaf = loadc("eaf", act_stage=True)
            c_e0 = loadc("e0")
            c_thi = loadc("t_hi", BF16, act_stage=True)
            c_tmid = loadc("t_mid", BF16, act_stage=True)
            c_tlo = loadc("t_lo", BF16, act_stage=True)
            c_iota = loadc("iota48")
            c_t47 = loadc("t47rep")

            # ---- state ----
            vfull = pspool.tile([128, 48], F32, tag="vfull", name="vfull")
            nc.scalar.copy(vfull[:], c_vinit[:])         # ACT writes PSUM
            e_st = spool.tile([48, NPC], F32, tag="e_st", name="e_st")
            nc.gpsimd.tensor_copy(e_st[:], c_e0[:])
            c_st = spool.tile([1, NPC], F32, tag="c_st", name="c_st")
            nc.gpsimd.memset(c_st[:], 0.0)

            # =================== FORWARD ===================
            ppool_cm = tc.tile_pool(name="pwf", bufs=2, space="PSUM")
            ppool = ppool_cm.__enter__()
            with tc.For_i(0, nchunk, 1) as iv:
                # featA[2b+h, k, i'] = feats[b, t0+k, 24h+i']
                featA = fpool.tile([128, ch, 24], F32, tag="featA", name="featA")
                nc.sync.dma_start(out=featA[0:NPC, :, :],
                                  in_=D["feats"][:, bass.ts(iv, ch), 0:24])
                nc.sync.dma_start(out=featA[NPC:128, :, :],
                                  in_=D["feats"][:, bass.ts(iv, ch), 24:48])
                featN = fpool.tile([NPC, ch, 48], F32, tag="featN", name="featN")
                nc.sync.dma_start(out=featN[:], in_=D["feats"][:, bass.ts(iv, ch), :])

                for k in range(ch):
                    # Viterbi P1 + P2
                    sc = wpool.tile([128, 24 * 48], F32, tag="sc", name="sc")
                    nc.vector.tensor_tensor(
                        apx(sc, [[48, 24], [1, 48]]),
                        apx(c_trep, [[48, 24], [1, 48]]),
                        apx(vfull, [[0, 24], [1, 48]]),
                        OP.add)
                    vmax = wpool.tile([128, 24], F32, tag="vmax", name="vmax")
                    nc.vector.tensor_reduce(
                        vmax[:], apx(sc, [[48, 24], [1, 48]]), axis=AX.X, op=OP.max)
                    # assembly: v_full = perm(vmax) + perm(feat)  (fp32 exact)
                    nc.tensor.matmul(vfull[:, 0:24], c_ident[:], vmax[:],
                                     start=True, stop=False)
                    nc.tensor.matmul(vfull[:, 0:24], c_pfi[:], featA[:, k, :],
                                     start=False, stop=True)
                    nc.tensor.matmul(vfull[:, 24:48], c_xor1[:], vmax[:],
                                     start=True, stop=False)
                    nc.tensor.matmul(vfull[:, 24:48], c_pfx[:], featA[:, k, :],
                                     start=False, stop=True)
                    # store v_t ([128, 0:24] == [b, 48] naturally)
                    vstg = wpool.tile([128, 24], F32, tag="vstg", name="vstg")
                    nc.scalar.copy(vstg[:], vfull[:, 0:24])
                    nc.sync.dma_start(
                        out=v_dram[bass.DynSlice(iv * ch + k, 1), :, :],
                        in_=vstg[:])

                    # ---- logZ ----
                    ftr = ppool.tile([48, NPC], F32, tag="ftr", name="ftr")
                    nc.tensor.transpose(ftr[:], featN[:, k, :], c_ident[:NPC, :NPC])
                    fhat = wpool.tile([48, NPC], F32, tag="fhat", name="fhat")
                    nc.scalar.activation(fhat[:], ftr[:], ACTF.Exp)
                    u_ps = ppool.tile([65, NPC], F32, tag="u_ps", name="u_ps")
                    nc.tensor.matmul(u_ps[:], c_ea[:], e_st[:], start=True, stop=True)
                    u_sb = wpool.tile([65, NPC], F32, tag="u_sb", name="u_sb")
                    nc.scalar.copy(u_sb[:], u_ps[:])
                    nc.gpsimd.tensor_tensor(e_st[:], fhat[:], u_sb[0:48, :], OP.mult)
                    if k % NORM_EVERY == NORM_EVERY - 1:
                        lns = wpool.tile([1, NPC], F32, tag="lns", name="lns")
                        nc.scalar.activation(lns[:], u_sb[64:65, :], ACTF.Ln)
                        rinv = wpool.tile([1, NPC], F32, tag="rinv", name="rinv")
                        nc.scalar.activation(rinv[:], lns[:], ACTF.Exp, scale=-1.0)
                        rrep = wpool.tile([48, NPC], F32, tag="rrep", name="rrep")
                        nc.gpsimd.partition_broadcast(rrep[:], rinv[:], channels=48)
                        nc.gpsimd.tensor_tensor(e_st[:], e_st[:], rrep[:], OP.mult)
                        nc.gpsimd.tensor_tensor(c_st[:], c_st[:], lns[:], OP.add)

            # logz final
            u_fin = ppool.tile([65, NPC], F32, tag="u_ps", name="u_fin")
            nc.tensor.matmul(u_fin[:], c_eaf[:], e_st[:], start=True, stop=True)
            u_fsb = spool.tile([65, NPC], F32, tag="u_fsb", name="u_fsb")
            nc.scalar.copy(u_fsb[:], u_fin[:])
            lzf = spool.tile([1, NPC], F32, tag="lzf", name="lzf")
            nc.scalar.activation(lzf[:], u_fsb[64:65, :], ACTF.Ln)
            lzrow = spool.tile([1, NPC], F32, tag="lzrow", name="lzrow")
            nc.vector.tensor_tensor(lzrow[:], lzf[:], c_st[:], OP.add)
            nc.sync.dma_start(out=D["logz"][:], in_=lzrow[:])

            # scores: max_j(v_{T-1}[b,j] + tb[END, j])
            vlast = spool.tile([NPC, 48], F32, tag="vlast", name="vlast")
            nc.sync.dma_start(out=vlast[:], in_=v_dram[t_total - 1, :, :])
            term = spool.tile([NPC, 48], F32, tag="term", name="term")
            nc.vector.tensor_tensor(term[:], vlast[:], c_t47[:], OP.add)
            m8s = spool.tile([NPC, 8], F32, tag="m8s", name="m8s")
            nc.vector.max(m8s[:], term[:])
            nc.sync.dma_start(out=D["scores"][:], in_=m8s[:, 0:1])

            ppool_cm.__exit__(None, None, None)

            # =================== BACKWARD ===================
            ppool_cm = tc.tile_pool(name="pwb", bufs=2, space="PSUM")
            ppool = ppool_cm.__enter__()
            idx_st = spool.tile([NPC, 8], U32, tag="idx_st", name="idx_st")
            nc.vector.memset(idx_st[:], END)

            with tc.For_i(0, nchunk, 1) as jv:
                cidx = (nchunk - 1) - jv
                vch = fpool.tile([NPC, ch, 48], F32, tag="vch", name="vch")
                vin = bass.AP(tensor=v_dram, offset=(cidx * ch) * (NPC * 48),
                              ap=[[48, NPC], [NPC * 48, ch], [1, 48]])
                nc.sync.dma_start(out=vch[:], in_=vin)
                pch = fpool.tile([NPC, ch], U32, tag="pch", name="pch")
                for kk in range(ch):
                    k = ch - 1 - kk      # m = cidx*ch + k, from high to low
                    tagf = wpool.tile([NPC, 1], F32, tag="tagf", name="tagf")
                    nc.gpsimd.tensor_copy(tagf[:], idx_st[:, 0:1])
                    oh = wpool.tile([NPC, 48], BF16, tag="oh", name="oh")
                    nc.gpsimd.tensor_scalar(
                        out=oh[:], in0=c_iota[:], scalar1=tagf[:, 0:1],
                        scalar2=None, op0=OP.is_equal)
                    ohp = ppool.tile([48, NPC], BF16, tag="ohp", name="ohp")
                    nc.tensor.transpose(ohp[:], oh[:], c_identb[:NPC, :NPC])
                    ohT = wpool.tile([48, NPC], BF16, tag="ohT", name="ohT")
                    nc.scalar.copy(ohT[:], ohp[:])
                    scp = ppool.tile([NPC, 48], F32, tag="scp", name="scp")
                    nc.tensor.matmul(scp[:], ohT[:], c_thi[:], start=True, stop=False)
                    nc.tensor.matmul(scp[:], ohT[:], c_tmid[:], start=False, stop=False)
                    nc.tensor.matmul(scp[:], ohT[:], c_tlo[:], start=False, stop=False)
                    nc.tensor.matmul(scp[:], c_i64[:], vch[:, k, :],
                                     start=False, stop=True)
                    scb = wpool.tile([NPC, 48], F32, tag="scb", name="scb")
                    nc.scalar.copy(scb[:], scp[:])
                    if _dbg:
                        nc.sync.dma_start(
                            out=D["dbg_sc"][bass.DynSlice(cidx * ch + k, 1), :, :],
                            in_=scb[:])
                        nc.sync.dma_start(
                            out=D["dbg_tag"][bass.DynSlice(cidx * ch + k, 1), :],
                            in_=tagf[:])
                    m8 = wpool.tile([NPC, 8], F32, tag="m8", name="m8")
                    nc.vector.max(m8[:], scb[:])
                    nc.vector.max_index(idx_st[:], m8[:], scb[:])
                    nc.gpsimd.tensor_copy(pch[:, k:k + 1], idx_st[:, 0:1])
                nc.sync.dma_start(
                    out=D["paths"][:, bass.DynSlice(cidx * ch, ch)], in_=pch[:])
            ppool_cm.__exit__(None, None, None)

    nc.finalize()
    return nc


def _get_nc(t_total=T, ch=CH):
    key = (t_total, ch)
    if key not in _CACHE:
        _CACHE[key] = _build(t_total, ch)
    return _CACHE[key]


def _run(feats, transitions, **kw):
    from concourse.bass_utils import run_bass_kernel_spmd

    feats = np.asarray(feats, dtype=np.float32)
    transitions = np.asarray(transitions, dtype=np.float32)
    t_total = feats.shape[1]
    nc = _get_nc(t_total=t_total, ch=CH if t_total % CH == 0 else t_total)

    consts = _consts(transitions)
    in_maps = []
    for c in range(NCORES):
        m = dict(consts)
        m["feats"] = np.ascontiguousarray(feats[c * NPC:(c + 1) * NPC])
        in_maps.append(m)

    bkr = run_bass_kernel_spmd(nc, in_maps, list(range(NCORES)), **kw)
    res = bkr.results
    logz = np.concatenate([r["logz"].reshape(-1) for r in res])
    scores = np.concatenate([r["scores"].reshape(-1) for r in res])
    paths = np.concatenate([r["paths"].astype(np.int32) for r in res], axis=0)
    return (logz.astype(np.float32), scores.astype(np.float32), paths), bkr


def kernel(feats, transitions):
    out, _ = _run(feats, transitions)
    return out
